# revision 51
# baseline (speedup 1.0000x reference)
"""AttentionPooling (segment softmax-pool) Trainium2 Bass kernel.

Full-input contract: kernel(**inputs) takes the unsharded inputs and
returns the full [1024, 256] float32 output. Internally shards 1024
graphs across 8 NeuronCores (128 contiguous graphs each, node ranges
padded to a common length) and runs one SPMD Bass/Tile kernel.

Math per core (one pass over x):
  h   = tanh((x8 @ W1_8) / 64 + b1)   # PE (fp8 e3m4) + ACT, [hidden, node]
  s   = h @ W2                        # PE, N=1 matmuls -> scores as columns
  e   = exp(s + b2)                   # ACT
  scat[i, seg] = (batchloc[i]==seg)*e # DVE tensor_scalar (is_equal, mult)
  acc[seg, 0:256] += scat.T @ x       # PE, PSUM accumulate across all tiles
  acc[seg, 256]   += scat.T @ 1       # fused via ones column of x_aug
  out[seg] = acc[seg, 0:256] / (acc[seg, 256] + 1e-8)

Numerics: the score path (x, W1) rides in fp8 e3m4 (x ~ N(0,1) fits the
+-15.5 range; W1 is pre-scaled by 64 on the host to clear e3m4's
subnormal floor and the tanh activation un-scales it), which halves the
HBM bytes of the transposed copy at ~6e-3 final rel-err. The output
path (xa, scat) must stay bf16: any fp8 there fails the 2e-2 gate.
Skipping the segment-max subtraction is safe: |s| <= ||W2||_1 + |b2|
(~12), so exp never overflows fp32.

Layout/DMA design (the kernel is HBM-bound end to end):
  - Two host-swizzled copies of x ship per core: xct (transposed,
    feature-on-partition, fp8, feeds the W1 matmuls) and xca (natural,
    node-on-partition, bf16 + ones column, feeds the scatter matmul).
    Every steady-state DMA is one contiguous [128, N] block per
    1024-node supergroup (2KB fp8 / 4.1KB bf16 per partition row).
  - ALL bulk DMAs issue on the sync (SP) HWDGE ring; putting them on
    the scalar ring stalls ACT behind pool-recycle waits (measured
    +26us). Output DMAs use the scalar ring (idle at finalize time).
  - Tiny packed consts ([w1], [iota|w2], [b1|b2|bl]) are issued first
    so the PE warm-up and first W1 matmul are never DMA-starved.
  - Deep prefetch (20 supergroups in flight, ~150KB/partition of SBUF)
    keeps the SDMA queue fed so the 16 engines stream gap-free at
    ~330 GB/s (the practical per-NC share of the HBM stack with both
    NCs active).
  - Node padding is 256-granular; an odd 512-node tail runs as a half
    supergroup. The last xa transfers land group-by-group so the final
    scatters overlap the last bytes in flight.
  - 24 warm-up matmuls on a memset tile (no DMA dependency) bridge the
    HAM clock-gate (K=4/8 -> 8/8) across first-data-arrival jitter.
Each core's 128 graphs are split into two 64-segment virtual shards
with separate PSUM accumulators, halving the scatter-matrix width (DVE
cost) and the scatter stationary loads. The loop is software-pipelined
with >=1 supergroup of distance between every producer/consumer pair.
"""

import os
from contextlib import ExitStack

import ml_dtypes
import numpy as np

N_CORES = 8
NUM_GRAPHS = 1024
BL = NUM_GRAPHS // N_CORES  # local segments per core = 128
HIDDEN = 256
HH = 128  # mlp hidden
P = 128
GROUP = 512  # nodes per compute group (4 tiles of 128)
SUPER = 1024  # nodes per DMA supergroup (2 compute groups)
XW = 257  # x row width in xa block: 256 features + ones col
SBL = 64  # segments per virtual shard (2 virtual shards per core)
XA_BLK = 4 * XW  # 1028 elems per compute group per partition


def _build_bass(npad: int):
    # npad is a multiple of GROUP (512); when npad % SUPER == 512 the last
    # supergroup is a half supergroup holding a single 512-node group.
    import concourse.bacc as bacc
    import concourse.mybir as mybir
    import concourse.tile as tile

    dt = mybir.dt
    G = npad // GROUP
    Gd = (npad + SUPER - 1) // SUPER
    TAIL = npad % SUPER != 0  # last supergroup has only group q=0
    T = npad // P

    nc = bacc.Bacc("TRN2", target_bir_lowering=False, debug=False)

    xct = nc.dram_tensor("xct", [Gd, P, 2 * SUPER], dt.float8e3, kind="ExternalInput")
    xca = nc.dram_tensor("xca", [Gd, P, 2 * XA_BLK], dt.bfloat16, kind="ExternalInput")
    w1 = nc.dram_tensor("w1", [P, 2 * HH], dt.float8e3, kind="ExternalInput")
    # packed consts: cst16 = [iota | w2], cst32 = [b1 | b2 | bl]
    cst16 = nc.dram_tensor("cst16", [P, SBL + 1], dt.bfloat16, kind="ExternalInput")
    cst32 = nc.dram_tensor("cst32", [P, 2 + T], dt.float32, kind="ExternalInput")
    out = nc.dram_tensor("out", [BL, HIDDEN], dt.float32, kind="ExternalOutput")

    with tile.TileContext(nc) as tc, ExitStack() as ctx:
        const = ctx.enter_context(tc.tile_pool(name="const", bufs=1))
        edge = ctx.enter_context(tc.tile_pool(name="edge", bufs=1))
        # xa buffers are fully unrolled (one per supergroup): an xa DMA issue
        # never waits on pool recycling, so the sync queue can never stall
        # the SDMA engines mid-stream on a compute hiccup.
        xt_pool = ctx.enter_context(tc.tile_pool(name="xt", bufs=22))
        xa_pool = ctx.enter_context(tc.tile_pool(name="xa", bufs=max(Gd, 2)))
        th_pool = ctx.enter_context(tc.tile_pool(name="th", bufs=4))
        e_pool = ctx.enter_context(tc.tile_pool(name="e", bufs=4))
        scat_pool = ctx.enter_context(tc.tile_pool(name="scat", bufs=32))
        fin_pool = ctx.enter_context(tc.tile_pool(name="fin", bufs=1))
        ph_pool = ctx.enter_context(tc.tile_pool(name="ph", bufs=2, space="PSUM"))
        ps_pool = ctx.enter_context(tc.tile_pool(name="ps", bufs=2, space="PSUM"))
        acc_pool = ctx.enter_context(tc.tile_pool(name="acc", bufs=1, space="PSUM"))

        # Head-of-stream: one bulk transfer (xa[0], not consumed until the
        # first scatter at dd=3) goes out first so the SDMA engines stream
        # while the remaining ~585ns-serialized issue instructions drain.
        xa0_t = edge.tile([P, 2 * XA_BLK], dt.bfloat16, tag="xa0")
        nc.sync.dma_start(xa0_t[:], xca[0])
        xt1_t = edge.tile([P, 2 * SUPER], dt.float8e3, tag="xt1")
        nc.sync.dma_start(xt1_t[:], xct[1])

        # then the tiny packed consts that unblock the PE warm-up and the
        # first real matmul long before the bulk x DMAs drain.
        w1_sb = const.tile([P, 2 * HH], dt.float8e3)
        nc.sync.dma_start(w1_sb[:], w1[:])
        c16_sb = const.tile([P, SBL + 1], dt.bfloat16)
        nc.sync.dma_start(c16_sb[:], cst16[:])
        c32_sb = const.tile([P, 2 + T], dt.float32)
        nc.sync.dma_start(c32_sb[:], cst32[:])
        iota_sb = c16_sb[:, 0:SBL]
        w2_sb = c16_sb[:, SBL : SBL + 1]
        b1_sb = c32_sb[:, 0:1]
        b2_sb = c32_sb[:, 1:2]

        # supergroup 0's host row is group-major ([h0 g0, h1 g0, h0 g1,
        # h1 g1]) so both halves are single contiguous runs.
        first_xt_c = edge.tile([P, 2, GROUP], dt.float8e3, tag="xtc0")
        nc.sync.dma_start(
            first_xt_c[:],
            xct[0][:, 0 : 2 * GROUP].rearrange("p (h n) -> p h n", h=2),
        )
        first_xt_r = edge.tile([P, 2, GROUP], dt.float8e3, tag="xtr0")
        nc.sync.dma_start(
            first_xt_r[:],
            xct[0][:, 2 * GROUP : 4 * GROUP].rearrange("p (h n) -> p h n", h=2),
        )

        acc_a = acc_pool.tile([SBL, XW], dt.float32)
        acc_b = acc_pool.tile([SBL, XW], dt.float32)
        t_half = (npad // 2) // P

        def finalize(k):
            # out = acc[:, 0:256] / acc[:, 256] for virtual shard k. The
            # reference's +1e-8 guard is numerically irrelevant here: every
            # graph has >=150 nodes and e >= exp(-|s|max) ~ 0.2, so the
            # denominator is always >= ~30.
            acc = (acc_a, acc_b)[k]
            recip = fin_pool.tile([SBL, 1], dt.float32, tag=f"rc{k}")
            nc.vector.reciprocal(recip[:], acc[:, HIDDEN : HIDDEN + 1])
            outf = fin_pool.tile([SBL, HIDDEN], dt.float32, tag=f"of{k}")
            nc.vector.tensor_scalar_mul(outf[:], acc[:, 0:HIDDEN], recip[:, 0:1])
            nc.scalar.dma_start(out[k * SBL : (k + 1) * SBL, :], outf[:])

        # PE warm-up: dummy matmuls on a memset tile (no DMA dependency at
        # all) start right after the runtime preamble and bring HAM to K=8/8
        # before the first real matmul.
        warm = const.tile([P, 2 * HH], dt.bfloat16)
        nc.vector.memset(warm[:], 0.0)
        for _ in range(24):
            wp = ph_pool.tile([HH, 2 * GROUP], dt.float32, tag="psum_h")
            nc.tensor.matmul(
                wp[:, 0 : 2 * HH], lhsT=warm[:, 0:HH], rhs=warm[:],
                start=True, stop=True,
            )

        n_tiles = G * 4
        xtts = {}
        xats = {}

        def dma_load_xt(d):
            if TAIL and d == Gd - 1:
                # tail host row is packed [h0 g0 | h1 g0] contiguously
                t = edge.tile([P, 2, GROUP], dt.float8e3, tag="xttail")
                nc.sync.dma_start(
                    t[:], xct[d][:, 0 : 2 * GROUP].rearrange("p (h n) -> p h n", h=2)
                )
            else:
                t = xt_pool.tile([P, 2 * SUPER], dt.float8e3)
                nc.sync.dma_start(t[:], xct[d])
            xtts[d] = t

        def dma_load_xa(d):
            if TAIL and d == Gd - 1:
                t = edge.tile([P, XA_BLK], dt.bfloat16, tag="xatail")
                nc.sync.dma_start(t[:], xca[d][:, 0:XA_BLK])
            else:
                t = xa_pool.tile([P, 2 * XA_BLK], dt.bfloat16)
                if d >= Gd - 3:
                    # near the stream tail, land each group separately so the
                    # final scatters overlap the last bytes in flight
                    nc.sync.dma_start(t[:, 0:XA_BLK], xca[d][:, 0:XA_BLK])
                    nc.sync.dma_start(t[:, XA_BLK : 2 * XA_BLK], xca[d][:, XA_BLK : 2 * XA_BLK])
                else:
                    nc.sync.dma_start(t[:], xca[d])
            xats[d] = t

        def xa_slice(g, s):
            t = xats[g // 2]
            if TAIL and g // 2 == Gd - 1:
                return t[:, s * XW : (s + 1) * XW]
            base = (g % 2) * XA_BLK + s * XW
            return t[:, base : base + XW]

        def xt_slice(g, h):
            d = g // 2
            if d == 0 or (TAIL and d == Gd - 1):
                if d == 0:
                    t = first_xt_c if g % 2 == 0 else first_xt_r
                else:
                    t = xtts[d]
                return t[:, h, :]
            t = xtts[d]
            base = h * SUPER + (g % 2) * GROUP
            return t[:, base : base + GROUP]

        ths = {}
        scats = {}

        # xt issue-lead over xa matches the 3-supergroup compute skew
        # (W1 consumes xt[dd] while the scatter consumes xa[dd-3]).
        XT_LEAD = 3
        PREFETCH = 20
        xtts[1] = xt1_t
        xats[0] = xa0_t
        for d in range(2, min(1 + XT_LEAD, Gd)):
            dma_load_xt(d)
        for k in range(PREFETCH):
            dt_ = 1 + XT_LEAD + k
            if dt_ < Gd:
                dma_load_xt(dt_)
            if 0 < k < Gd:
                dma_load_xa(k)

        for dd in range(Gd + 3):
            d_t = dd + 1 + XT_LEAD + PREFETCH
            if d_t < Gd:
                dma_load_xt(d_t)
            d_a = dd + PREFETCH
            if d_a < Gd:
                dma_load_xa(d_a)

            def qs_of(d):
                return (0,) if (TAIL and d == Gd - 1) else (0, 1)

            if dd < Gd:
                nq = len(qs_of(dd))
                psum_h = ph_pool.tile([HH, 2 * GROUP], dt.float32, tag="psum_h")
                for q in qs_of(dd):
                    g = 2 * dd + q
                    sl = slice(q * GROUP, (q + 1) * GROUP)
                    nc.tensor.matmul(
                        psum_h[:, sl], lhsT=w1_sb[:, 0:HH], rhs=xt_slice(g, 0),
                        start=True, stop=False,
                    )
                    nc.tensor.matmul(
                        psum_h[:, sl], lhsT=w1_sb[:, HH : 2 * HH], rhs=xt_slice(g, 1),
                        start=False, stop=True,
                    )
                th = th_pool.tile([HH, 2 * GROUP], dt.bfloat16)
                nc.scalar.activation(
                    th[:, 0 : nq * GROUP], psum_h[:, 0 : nq * GROUP],
                    mybir.ActivationFunctionType.Tanh,
                    bias=b1_sb, scale=1.0 / 64.0,
                )
                ths[dd] = th

            if 1 <= dd <= Gd:
                d1 = dd - 1
                th = ths.pop(d1)
                ns = 4 * len(qs_of(d1))
                psum_s = ps_pool.tile([P, 8], dt.float32)
                for si in range(ns):
                    nc.tensor.matmul(
                        psum_s[:, si : si + 1],
                        lhsT=th[:, si * P : (si + 1) * P],
                        rhs=w2_sb,
                        start=True, stop=True,
                    )
                e8 = e_pool.tile([P, 8], dt.float32)
                nc.scalar.activation(
                    e8[:, 0:ns], psum_s[:, 0:ns], mybir.ActivationFunctionType.Exp,
                    bias=b2_sb, scale=1.0,
                )
                for q in qs_of(d1):
                    g = 2 * d1 + q
                    row = []
                    for sx in range(4):
                        t = g * 4 + sx
                        scat = scat_pool.tile([P, SBL], dt.bfloat16)
                        nc.vector.tensor_scalar(
                            out=scat[:],
                            in0=iota_sb,
                            scalar1=c32_sb[:, 2 + t : 3 + t],
                            scalar2=e8[:, q * 4 + sx : q * 4 + sx + 1],
                            op0=mybir.AluOpType.is_equal,
                            op1=mybir.AluOpType.mult,
                        )
                        row.append(scat)
                    scats[g] = row

            if 3 <= dd:
                d2 = dd - 3
                for q in qs_of(d2):
                    g = 2 * d2 + q
                    row = scats.pop(g)
                    for s in range(4):
                        t = g * 4 + s
                        acc = acc_a if t < t_half else acc_b
                        nc.tensor.matmul(
                            acc[:],
                            lhsT=row[s][:],
                            rhs=xa_slice(g, s),
                            start=(t == 0 or t == t_half),
                            stop=(t == t_half - 1 or t == n_tiles - 1),
                            skip_group_check=True,
                        )
                if d2 == t_half // 8:
                    finalize(0)

        finalize(1)

    nc.compile()
    return nc


def _maybe_enable_trace():
    """Dev-only NTFF profiling: register the axon NTFF hook if available.
    Inert when ATT_POOL_TRACE is unset (the grading path)."""
    if os.environ.get("ATT_POOL_TRACE") != "1":
        return False
    try:
        import sys
        import types

        import trn_agent_boot.trn_boot as tb

        hook = tb._ntff_profile_via_ctypes("/opt/axon/libaxon_pjrt.so")
        mod = types.ModuleType("antenv.axon_hooks")
        mod.get_axon_ntff_profile_hook = lambda: hook
        mod.set_axon_ntff_profile_hook = lambda h: None
        sys.modules["antenv.axon_hooks"] = mod

        import concourse.bass_utils as bu

        bu.upload_artifacts = lambda tmpdir: "local://" + str(tmpdir)
        return True
    except Exception as e:  # pragma: no cover - dev path only
        print("trace setup failed:", e)
        return False


def kernel(x, batch, W1, b1, W2, b2):
    from concourse.bass_utils import run_bass_kernel_spmd

    x = np.asarray(x, dtype=np.float32)
    batch = np.asarray(batch).astype(np.int64)
    W1 = np.asarray(W1, dtype=np.float32)
    b1 = np.asarray(b1, dtype=np.float32)
    W2 = np.asarray(W2, dtype=np.float32)
    b2 = np.asarray(b2, dtype=np.float32)

    bf16 = ml_dtypes.bfloat16
    f8 = ml_dtypes.float8_e3m4

    bounds = np.searchsorted(batch, np.arange(0, NUM_GRAPHS + 1, SBL))
    shard = np.diff(bounds)
    npad_half = int(-(-int(shard.max()) // 256) * 256)
    npad = 2 * npad_half  # multiple of 512; may end in a half supergroup
    G = npad // GROUP
    Gd = (npad + SUPER - 1) // SUPER
    nfull = Gd * SUPER  # host staging is sized in full supergroups
    T = npad // P

    x_bf = x.astype(bf16)
    x_f8 = x.astype(f8)
    xct_h = np.zeros((N_CORES, Gd, P, 2 * SUPER), dtype=f8)
    xca_h = np.zeros((N_CORES, Gd, P, 2 * XA_BLK), dtype=bf16)
    blh = np.full((N_CORES, P, T), 300.0, dtype=np.float32)
    for c in range(N_CORES):
        xa = np.zeros((nfull, XW), dtype=bf16)
        xt = np.zeros((2, P, nfull), dtype=f8)
        bl = np.full(nfull, 300.0, dtype=np.float32)
        for hh in range(2):
            idx = 2 * c + hh
            s0, s1 = int(bounds[idx]), int(bounds[idx + 1])
            n = s1 - s0
            o = hh * npad_half
            xa[o : o + n, :HIDDEN] = x_bf[s0:s1]
            xa[o : o + n, HIDDEN] = 1.0
            xt[0, :, o : o + n] = x_f8[s0:s1, 0:P].T
            xt[1, :, o : o + n] = x_f8[s0:s1, P:HIDDEN].T
            bl[o : o + n] = (batch[s0:s1] - idx * SBL).astype(np.float32)
        xa_sw = (
            xa.reshape(Gd, 2 * 4 * P * XW)
            .reshape(Gd, 2, 4, P, XW)
            .transpose(0, 3, 1, 2, 4)
            .reshape(Gd, P, 2 * XA_BLK)
        )
        xca_h[c] = xa_sw
        xt_sw = (
            xt.reshape(2, P, Gd, SUPER).transpose(2, 1, 0, 3).reshape(Gd, P, 2 * SUPER)
        )
        # supergroup 0: group-major packing [h0 g0 | h1 g0 | h0 g1 | h1 g1]
        xt_sw[0] = (
            xt_sw[0]
            .reshape(P, 2, 2, GROUP)
            .transpose(0, 2, 1, 3)
            .reshape(P, 2 * SUPER)
        )
        if npad % SUPER:
            # tail supergroup: pack the single valid group [h0 g0 | h1 g0]
            tl = xt_sw[Gd - 1].reshape(P, 2, SUPER)[:, :, 0:GROUP].copy()
            xt_sw[Gd - 1] = 0
            xt_sw[Gd - 1, :, 0 : 2 * GROUP] = tl.reshape(P, 2 * GROUP)
        xct_h[c] = xt_sw
        blh[c] = bl[:npad].reshape(T, P).T
    # w1 packed [P, 2*HH]: feature-half h at columns [h*HH, (h+1)*HH)
    w1_pk = np.ascontiguousarray(
        (W1 * 64.0).astype(f8).reshape(2, P, HH).transpose(1, 0, 2).reshape(P, 2 * HH)
    )
    # cst16 = [iota | w2], cst32 = [b1 | b2 | bl]
    c16 = np.zeros((P, SBL + 1), dtype=bf16)
    c16[:, 0:SBL] = np.arange(SBL, dtype=np.float32)[None, :].astype(bf16)
    c16[:, SBL] = W2.astype(bf16).reshape(HH)
    c32 = np.zeros((N_CORES, P, 2 + T), dtype=np.float32)
    c32[:, :, 0] = b1.reshape(1, HH)
    c32[:, :, 1] = float(np.asarray(b2).reshape(-1)[0])
    c32[:, :, 2:] = blh

    trace = _maybe_enable_trace()
    nc = _build_bass(npad)

    in_maps = []
    for c in range(N_CORES):
        in_maps.append(
            {
                "xct": xct_h[c],
                "xca": xca_h[c],
                "w1": w1_pk,
                "cst16": c16,
                "cst32": c32[c],
            }
        )

    res = run_bass_kernel_spmd(
        nc, in_maps, core_ids=list(range(N_CORES)), trace=trace
    )
    if trace and res.exec_time_ns is not None:
        print(f"HW exec time: {res.exec_time_ns} ns")
        if res.instructions_and_trace:
            print("trace:", res.instructions_and_trace[1])
        if res.profile_json:
            print("profile_json:", res.profile_json)

    out = np.concatenate([res.results[c]["out"] for c in range(N_CORES)], axis=0)
    assert out.shape == (NUM_GRAPHS, HIDDEN)
    return np.ascontiguousarray(out.astype(np.float32))



# revision 52
# speedup vs baseline: 1.0124x; 1.0124x over previous
"""AttentionPooling (segment softmax-pool) Trainium2 Bass kernel.

Full-input contract: kernel(**inputs) takes the unsharded inputs and
returns the full [1024, 256] float32 output. Internally shards 1024
graphs across 8 NeuronCores (128 contiguous graphs each, node ranges
padded to a common length) and runs one SPMD Bass/Tile kernel.

Math per core (one pass over x):
  h   = tanh((x8 @ W1_8) / 64 + b1)   # PE (fp8 e3m4) + ACT, [hidden, node]
  s   = h @ W2                        # PE, N=1 matmuls -> scores as columns
  e   = exp(s + b2)                   # ACT
  scat[i, seg] = (batchloc[i]==seg)*e # DVE tensor_scalar (is_equal, mult)
  acc[seg, 0:256] += scat.T @ x       # PE, PSUM accumulate across all tiles
  acc[seg, 256]   += scat.T @ 1       # fused via ones column of x_aug
  out[seg] = acc[seg, 0:256] / (acc[seg, 256] + 1e-8)

Numerics: the score path (x, W1) rides in fp8 e3m4 (x ~ N(0,1) fits the
+-15.5 range; W1 is pre-scaled by 64 on the host to clear e3m4's
subnormal floor and the tanh activation un-scales it), which halves the
HBM bytes of the transposed copy at ~6e-3 final rel-err. The output
path (xa, scat) must stay bf16: any fp8 there fails the 2e-2 gate.
Skipping the segment-max subtraction is safe: |s| <= ||W2||_1 + |b2|
(~12), so exp never overflows fp32.

Layout/DMA design (the kernel is HBM-bound end to end):
  - Two host-swizzled copies of x ship per core: xct (transposed,
    feature-on-partition, fp8, feeds the W1 matmuls) and xca (natural,
    node-on-partition, bf16 + ones column, feeds the scatter matmul).
    Every steady-state DMA is one contiguous [128, N] block per
    1024-node supergroup (2KB fp8 / 4.1KB bf16 per partition row).
  - ALL bulk DMAs issue on the sync (SP) HWDGE ring; putting them on
    the scalar ring stalls ACT behind pool-recycle waits (measured
    +26us). Output DMAs use the scalar ring (idle at finalize time).
  - Tiny packed consts ([w1], [iota|w2], [b1|b2|bl]) are issued first
    so the PE warm-up and first W1 matmul are never DMA-starved.
  - Deep prefetch (20 supergroups in flight, ~150KB/partition of SBUF)
    keeps the SDMA queue fed so the 16 engines stream gap-free at
    ~330 GB/s (the practical per-NC share of the HBM stack with both
    NCs active).
  - Node padding is 256-granular; an odd 512-node tail runs as a half
    supergroup. The last xa transfers land group-by-group so the final
    scatters overlap the last bytes in flight.
  - 24 warm-up matmuls on a memset tile (no DMA dependency) bridge the
    HAM clock-gate (K=4/8 -> 8/8) across first-data-arrival jitter.
Each core's 128 graphs are split into two 64-segment virtual shards
with separate PSUM accumulators, halving the scatter-matrix width (DVE
cost) and the scatter stationary loads. The loop is software-pipelined
with >=1 supergroup of distance between every producer/consumer pair.
"""

import os
from contextlib import ExitStack

import ml_dtypes
import numpy as np

N_CORES = 8
NUM_GRAPHS = 1024
BL = NUM_GRAPHS // N_CORES  # local segments per core = 128
HIDDEN = 256
HH = 128  # mlp hidden
P = 128
GROUP = 512  # nodes per compute group (4 tiles of 128)
SUPER = 1024  # nodes per DMA supergroup (2 compute groups)
XW = 257  # x row width in xa block: 256 features + ones col
SBL = 64  # segments per virtual shard (2 virtual shards per core)
XA_BLK = 4 * XW  # 1028 elems per compute group per partition


def _build_bass(npad: int):
    # npad is a multiple of GROUP (512); when npad % SUPER == 512 the last
    # supergroup is a half supergroup holding a single 512-node group.
    import concourse.bacc as bacc
    import concourse.mybir as mybir
    import concourse.tile as tile

    dt = mybir.dt
    G = npad // GROUP
    Gd = (npad + SUPER - 1) // SUPER
    TAIL = npad % SUPER != 0  # last supergroup has only group q=0
    T = npad // P

    nc = bacc.Bacc("TRN2", target_bir_lowering=False, debug=False)

    xct = nc.dram_tensor("xct", [Gd, P, 2 * SUPER], dt.float8e3, kind="ExternalInput")
    xca = nc.dram_tensor("xca", [Gd, P, 2 * XA_BLK], dt.bfloat16, kind="ExternalInput")
    w1 = nc.dram_tensor("w1", [P, 2 * HH], dt.float8e3, kind="ExternalInput")
    # packed consts: cst16 = [iota | w2], cst32 = [b1 | b2 | bl]
    cst16 = nc.dram_tensor("cst16", [P, SBL + 1], dt.bfloat16, kind="ExternalInput")
    cst32 = nc.dram_tensor("cst32", [P, 2 + T], dt.float32, kind="ExternalInput")
    out = nc.dram_tensor("out", [BL, HIDDEN], dt.float32, kind="ExternalOutput")

    with tile.TileContext(nc) as tc, ExitStack() as ctx:
        const = ctx.enter_context(tc.tile_pool(name="const", bufs=1))
        edge = ctx.enter_context(tc.tile_pool(name="edge", bufs=1))
        # xa buffers are fully unrolled (one per supergroup): an xa DMA issue
        # never waits on pool recycling, so the sync queue can never stall
        # the SDMA engines mid-stream on a compute hiccup.
        xt_pool = ctx.enter_context(tc.tile_pool(name="xt", bufs=20))
        xa_pool = ctx.enter_context(tc.tile_pool(name="xa", bufs=max(Gd, 2)))
        th_pool = ctx.enter_context(tc.tile_pool(name="th", bufs=4))
        e_pool = ctx.enter_context(tc.tile_pool(name="e", bufs=4))
        scat_pool = ctx.enter_context(tc.tile_pool(name="scat", bufs=32))
        fin_pool = ctx.enter_context(tc.tile_pool(name="fin", bufs=1))
        ph_pool = ctx.enter_context(tc.tile_pool(name="ph", bufs=2, space="PSUM"))
        ps_pool = ctx.enter_context(tc.tile_pool(name="ps", bufs=2, space="PSUM"))
        acc_pool = ctx.enter_context(tc.tile_pool(name="acc", bufs=1, space="PSUM"))

        # Head-of-stream: one bulk transfer (xa[0], not consumed until the
        # first scatter at dd=3) goes out first so the SDMA engines stream
        # while the remaining ~585ns-serialized issue instructions drain.
        xa0_t = edge.tile([P, 2 * XA_BLK], dt.bfloat16, tag="xa0")
        nc.sync.dma_start(xa0_t[:], xca[0])
        xt1_t = edge.tile([P, 2 * SUPER], dt.float8e3, tag="xt1")
        nc.sync.dma_start(xt1_t[:], xct[1])

        # then the tiny packed consts that unblock the PE warm-up and the
        # first real matmul long before the bulk x DMAs drain.
        w1_sb = const.tile([P, 2 * HH], dt.float8e3)
        nc.sync.dma_start(w1_sb[:], w1[:])
        c16_sb = const.tile([P, SBL + 1], dt.bfloat16)
        nc.sync.dma_start(c16_sb[:], cst16[:])
        c32_sb = const.tile([P, 2 + T], dt.float32)
        nc.sync.dma_start(c32_sb[:], cst32[:])
        iota_sb = c16_sb[:, 0:SBL]
        w2_sb = c16_sb[:, SBL : SBL + 1]
        b1_sb = c32_sb[:, 0:1]
        b2_sb = c32_sb[:, 1:2]

        # supergroup 0's host row is group-major ([h0 g0, h1 g0, h0 g1,
        # h1 g1]) so both halves are single contiguous runs.
        first_xt_c = edge.tile([P, 2, GROUP], dt.float8e3, tag="xtc0")
        nc.sync.dma_start(
            first_xt_c[:],
            xct[0][:, 0 : 2 * GROUP].rearrange("p (h n) -> p h n", h=2),
        )
        first_xt_r = edge.tile([P, 2, GROUP], dt.float8e3, tag="xtr0")
        nc.sync.dma_start(
            first_xt_r[:],
            xct[0][:, 2 * GROUP : 4 * GROUP].rearrange("p (h n) -> p h n", h=2),
        )

        acc_a = acc_pool.tile([SBL, XW], dt.float32)
        acc_b = acc_pool.tile([SBL, XW], dt.float32)
        t_half = (npad // 2) // P

        def finalize(k):
            # out = acc[:, 0:256] / acc[:, 256] for virtual shard k. The
            # reference's +1e-8 guard is numerically irrelevant here: every
            # graph has >=150 nodes and e >= exp(-|s|max) ~ 0.2, so the
            # denominator is always >= ~30.
            acc = (acc_a, acc_b)[k]
            recip = fin_pool.tile([SBL, 1], dt.float32, tag=f"rc{k}")
            nc.vector.reciprocal(recip[:], acc[:, HIDDEN : HIDDEN + 1])
            outf = fin_pool.tile([SBL, HIDDEN], dt.float32, tag=f"of{k}")
            nc.vector.tensor_scalar_mul(outf[:], acc[:, 0:HIDDEN], recip[:, 0:1])
            nc.scalar.dma_start(out[k * SBL : (k + 1) * SBL, :], outf[:])

        # PE warm-up: dummy matmuls on a memset tile (no DMA dependency at
        # all) start right after the runtime preamble and bring HAM to K=8/8
        # before the first real matmul.
        warm = const.tile([P, 2 * HH], dt.bfloat16)
        nc.vector.memset(warm[:], 0.0)
        for _ in range(24):
            wp = ph_pool.tile([HH, 2 * GROUP], dt.float32, tag="psum_h")
            nc.tensor.matmul(
                wp[:, 0 : 2 * HH], lhsT=warm[:, 0:HH], rhs=warm[:],
                start=True, stop=True,
            )

        n_tiles = G * 4
        xtts = {}
        xats = {}

        def dma_load_xt(d):
            if TAIL and d == Gd - 1:
                # tail host row is packed [h0 g0 | h1 g0] contiguously
                t = edge.tile([P, 2, GROUP], dt.float8e3, tag="xttail")
                nc.sync.dma_start(
                    t[:], xct[d][:, 0 : 2 * GROUP].rearrange("p (h n) -> p h n", h=2)
                )
            else:
                t = xt_pool.tile([P, 2 * SUPER], dt.float8e3)
                nc.sync.dma_start(t[:], xct[d])
            xtts[d] = t

        def dma_load_xa(d):
            if TAIL and d == Gd - 1:
                t = edge.tile([P, XA_BLK], dt.bfloat16, tag="xatail")
                nc.sync.dma_start(t[:], xca[d][:, 0:XA_BLK])
            else:
                t = xa_pool.tile([P, 2 * XA_BLK], dt.bfloat16)
                if d >= Gd - 3:
                    # near the stream tail, land each group separately so the
                    # final scatters overlap the last bytes in flight
                    nc.sync.dma_start(t[:, 0:XA_BLK], xca[d][:, 0:XA_BLK])
                    nc.sync.dma_start(t[:, XA_BLK : 2 * XA_BLK], xca[d][:, XA_BLK : 2 * XA_BLK])
                else:
                    nc.sync.dma_start(t[:], xca[d])
            xats[d] = t

        def xa_slice(g, s):
            t = xats[g // 2]
            if TAIL and g // 2 == Gd - 1:
                return t[:, s * XW : (s + 1) * XW]
            base = (g % 2) * XA_BLK + s * XW
            return t[:, base : base + XW]

        def xt_slice(g, h):
            d = g // 2
            if d == 0 or (TAIL and d == Gd - 1):
                if d == 0:
                    t = first_xt_c if g % 2 == 0 else first_xt_r
                else:
                    t = xtts[d]
                return t[:, h, :]
            t = xtts[d]
            base = h * SUPER + (g % 2) * GROUP
            return t[:, base : base + GROUP]

        ths = {}
        scats = {}

        # xt issue-lead over xa matches the 3-supergroup compute skew
        # (W1 consumes xt[dd] while the scatter consumes xa[dd-3]).
        XT_LEAD = 3
        PREFETCH = 20
        xtts[1] = xt1_t
        xats[0] = xa0_t
        for d in range(2, min(1 + XT_LEAD, Gd)):
            dma_load_xt(d)
        for k in range(PREFETCH):
            dt_ = 1 + XT_LEAD + k
            if dt_ < Gd:
                dma_load_xt(dt_)
            if 0 < k < Gd:
                dma_load_xa(k)

        for dd in range(Gd + 3):
            d_t = dd + 1 + XT_LEAD + PREFETCH
            if d_t < Gd:
                dma_load_xt(d_t)
            d_a = dd + PREFETCH
            if d_a < Gd:
                dma_load_xa(d_a)

            def qs_of(d):
                return (0,) if (TAIL and d == Gd - 1) else (0, 1)

            if dd < Gd:
                nq = len(qs_of(dd))
                psum_h = ph_pool.tile([HH, 2 * GROUP], dt.float32, tag="psum_h")
                for q in qs_of(dd):
                    g = 2 * dd + q
                    sl = slice(q * GROUP, (q + 1) * GROUP)
                    nc.tensor.matmul(
                        psum_h[:, sl], lhsT=w1_sb[:, 0:HH], rhs=xt_slice(g, 0),
                        start=True, stop=False,
                    )
                    nc.tensor.matmul(
                        psum_h[:, sl], lhsT=w1_sb[:, HH : 2 * HH], rhs=xt_slice(g, 1),
                        start=False, stop=True,
                    )
                th = th_pool.tile([HH, 2 * GROUP], dt.bfloat16)
                nc.scalar.activation(
                    th[:, 0 : nq * GROUP], psum_h[:, 0 : nq * GROUP],
                    mybir.ActivationFunctionType.Tanh,
                    bias=b1_sb, scale=1.0 / 64.0,
                )
                ths[dd] = th

            if 1 <= dd <= Gd:
                d1 = dd - 1
                th = ths.pop(d1)
                ns = 4 * len(qs_of(d1))
                psum_s = ps_pool.tile([P, 8], dt.float32)
                for si in range(ns):
                    nc.tensor.matmul(
                        psum_s[:, si : si + 1],
                        lhsT=th[:, si * P : (si + 1) * P],
                        rhs=w2_sb,
                        start=True, stop=True,
                    )
                e8 = e_pool.tile([P, 8], dt.float32)
                nc.scalar.activation(
                    e8[:, 0:ns], psum_s[:, 0:ns], mybir.ActivationFunctionType.Exp,
                    bias=b2_sb, scale=1.0,
                )
                for q in qs_of(d1):
                    g = 2 * d1 + q
                    row = []
                    for sx in range(4):
                        t = g * 4 + sx
                        scat = scat_pool.tile([P, SBL], dt.bfloat16)
                        nc.vector.tensor_scalar(
                            out=scat[:],
                            in0=iota_sb,
                            scalar1=c32_sb[:, 2 + t : 3 + t],
                            scalar2=e8[:, q * 4 + sx : q * 4 + sx + 1],
                            op0=mybir.AluOpType.is_equal,
                            op1=mybir.AluOpType.mult,
                        )
                        row.append(scat)
                    scats[g] = row

            if 3 <= dd:
                d2 = dd - 3
                for q in qs_of(d2):
                    g = 2 * d2 + q
                    row = scats.pop(g)
                    for s in range(4):
                        t = g * 4 + s
                        acc = acc_a if t < t_half else acc_b
                        nc.tensor.matmul(
                            acc[:],
                            lhsT=row[s][:],
                            rhs=xa_slice(g, s),
                            start=(t == 0 or t == t_half),
                            stop=(t == t_half - 1 or t == n_tiles - 1),
                            skip_group_check=True,
                        )
                if d2 == t_half // 8:
                    finalize(0)

        finalize(1)

    nc.compile()
    return nc


def _maybe_enable_trace():
    """Dev-only NTFF profiling: register the axon NTFF hook if available.
    Inert when ATT_POOL_TRACE is unset (the grading path)."""
    if os.environ.get("ATT_POOL_TRACE") != "1":
        return False
    try:
        import sys
        import types

        import trn_agent_boot.trn_boot as tb

        hook = tb._ntff_profile_via_ctypes("/opt/axon/libaxon_pjrt.so")
        mod = types.ModuleType("antenv.axon_hooks")
        mod.get_axon_ntff_profile_hook = lambda: hook
        mod.set_axon_ntff_profile_hook = lambda h: None
        sys.modules["antenv.axon_hooks"] = mod

        import concourse.bass_utils as bu

        bu.upload_artifacts = lambda tmpdir: "local://" + str(tmpdir)
        return True
    except Exception as e:  # pragma: no cover - dev path only
        print("trace setup failed:", e)
        return False


def kernel(x, batch, W1, b1, W2, b2):
    from concourse.bass_utils import run_bass_kernel_spmd

    x = np.asarray(x, dtype=np.float32)
    batch = np.asarray(batch).astype(np.int64)
    W1 = np.asarray(W1, dtype=np.float32)
    b1 = np.asarray(b1, dtype=np.float32)
    W2 = np.asarray(W2, dtype=np.float32)
    b2 = np.asarray(b2, dtype=np.float32)

    bf16 = ml_dtypes.bfloat16
    f8 = ml_dtypes.float8_e3m4

    bounds = np.searchsorted(batch, np.arange(0, NUM_GRAPHS + 1, SBL))
    shard = np.diff(bounds)
    npad_half = int(-(-int(shard.max()) // 256) * 256)
    npad = 2 * npad_half  # multiple of 512; may end in a half supergroup
    G = npad // GROUP
    Gd = (npad + SUPER - 1) // SUPER
    nfull = Gd * SUPER  # host staging is sized in full supergroups
    T = npad // P

    x_bf = x.astype(bf16)
    x_f8 = x.astype(f8)
    xct_h = np.zeros((N_CORES, Gd, P, 2 * SUPER), dtype=f8)
    xca_h = np.zeros((N_CORES, Gd, P, 2 * XA_BLK), dtype=bf16)
    blh = np.full((N_CORES, P, T), 300.0, dtype=np.float32)
    for c in range(N_CORES):
        xa = np.zeros((nfull, XW), dtype=bf16)
        xt = np.zeros((2, P, nfull), dtype=f8)
        bl = np.full(nfull, 300.0, dtype=np.float32)
        for hh in range(2):
            idx = 2 * c + hh
            s0, s1 = int(bounds[idx]), int(bounds[idx + 1])
            n = s1 - s0
            o = hh * npad_half
            xa[o : o + n, :HIDDEN] = x_bf[s0:s1]
            xa[o : o + n, HIDDEN] = 1.0
            xt[0, :, o : o + n] = x_f8[s0:s1, 0:P].T
            xt[1, :, o : o + n] = x_f8[s0:s1, P:HIDDEN].T
            bl[o : o + n] = (batch[s0:s1] - idx * SBL).astype(np.float32)
        xa_sw = (
            xa.reshape(Gd, 2 * 4 * P * XW)
            .reshape(Gd, 2, 4, P, XW)
            .transpose(0, 3, 1, 2, 4)
            .reshape(Gd, P, 2 * XA_BLK)
        )
        xca_h[c] = xa_sw
        xt_sw = (
            xt.reshape(2, P, Gd, SUPER).transpose(2, 1, 0, 3).reshape(Gd, P, 2 * SUPER)
        )
        # supergroup 0: group-major packing [h0 g0 | h1 g0 | h0 g1 | h1 g1]
        xt_sw[0] = (
            xt_sw[0]
            .reshape(P, 2, 2, GROUP)
            .transpose(0, 2, 1, 3)
            .reshape(P, 2 * SUPER)
        )
        if npad % SUPER:
            # tail supergroup: pack the single valid group [h0 g0 | h1 g0]
            tl = xt_sw[Gd - 1].reshape(P, 2, SUPER)[:, :, 0:GROUP].copy()
            xt_sw[Gd - 1] = 0
            xt_sw[Gd - 1, :, 0 : 2 * GROUP] = tl.reshape(P, 2 * GROUP)
        xct_h[c] = xt_sw
        blh[c] = bl[:npad].reshape(T, P).T
    # w1 packed [P, 2*HH]: feature-half h at columns [h*HH, (h+1)*HH)
    w1_pk = np.ascontiguousarray(
        (W1 * 64.0).astype(f8).reshape(2, P, HH).transpose(1, 0, 2).reshape(P, 2 * HH)
    )
    # cst16 = [iota | w2], cst32 = [b1 | b2 | bl]
    c16 = np.zeros((P, SBL + 1), dtype=bf16)
    c16[:, 0:SBL] = np.arange(SBL, dtype=np.float32)[None, :].astype(bf16)
    c16[:, SBL] = W2.astype(bf16).reshape(HH)
    c32 = np.zeros((N_CORES, P, 2 + T), dtype=np.float32)
    c32[:, :, 0] = b1.reshape(1, HH)
    c32[:, :, 1] = float(np.asarray(b2).reshape(-1)[0])
    c32[:, :, 2:] = blh

    trace = _maybe_enable_trace()
    nc = _build_bass(npad)

    in_maps = []
    for c in range(N_CORES):
        in_maps.append(
            {
                "xct": xct_h[c],
                "xca": xca_h[c],
                "w1": w1_pk,
                "cst16": c16,
                "cst32": c32[c],
            }
        )

    res = run_bass_kernel_spmd(
        nc, in_maps, core_ids=list(range(N_CORES)), trace=trace
    )
    if trace and res.exec_time_ns is not None:
        print(f"HW exec time: {res.exec_time_ns} ns")
        if res.instructions_and_trace:
            print("trace:", res.instructions_and_trace[1])
        if res.profile_json:
            print("profile_json:", res.profile_json)

    out = np.concatenate([res.results[c]["out"] for c in range(N_CORES)], axis=0)
    assert out.shape == (NUM_GRAPHS, HIDDEN)
    return np.ascontiguousarray(out.astype(np.float32))

